# revision 15
# baseline (speedup 1.0000x reference)
"""Trainium2 Bass kernel for nn_MGCNLoss (segment_reduce), v3.1.

Strategy (8 NeuronCores, SPMD, data-parallel over graphs):
  * Host routes each node to the core owning its graph and lays the core's
    nodes out as a dense zero-padded [128 partitions, F] fp16 matrix:
    partition p of supertile s holds one whole graph per (s, p) slot.
    Graphs are assigned to supertiles sorted by node count so each
    supertile's pad is tight (and the smallest supertile runs last to
    shorten the tail).
  * Normalize-first formulation, half-scaled: spH = xp/(sum xp+e)*SCL/2,
    snH likewise, u2 = spH+snH. One merged Ln pass per supertile over the
    contiguous [spH|snH|u2] block computes ln(2*v+b), one merged fp16
    tensor_tensor pass forms v*ln(2*v+b), and the idle PE engine reduces
    everything into a single PSUM [1,512] row with +1 weights for the
    sp/sn segments and -1 for the u segment:
        kl_total = 2*R/SCL + 2*ln2*num_graphs   (exact; validated 2e-5)
    No per-graph KL reductions exist at all; only the two normalization
    sums do (Scalar engine Copy+accum for xp, Vector fold-twice + short
    accum for xn), balancing both engine pipelines.
  * Cross-entropy ships per-graph sum(exp(logits)) to the host (no device
    ln => no activation-table switch mid-pipeline); the host, which
    already gathered the target logits during packing, finishes
    ce = sum(ln(se)) - sum(pick). MSE is one fused pass.
"""

import os

import numpy as np

import concourse.bass as bass
import concourse.bacc as bacc
import concourse.mybir as mybir
from concourse import tile
from concourse.bass_utils import run_bass_kernel_spmd

F32 = mybir.dt.float32
F16 = mybir.dt.float16
ALU = mybir.AluOpType
ACTF = mybir.ActivationFunctionType
AX = mybir.AxisListType

NUM_GRAPHS = 4096
NUM_NODES = 8_388_608
NUM_CLASSES = 10
NCORES = 8
GPC = NUM_GRAPHS // NCORES  # graphs per core = 512
ST = GPC // 128  # supertiles per core = 4
EPS = 1e-8
SCL = 1024.0  # fp16 anti-subnormal scale; global factors undone on host
ALPHA = 1.0
BETA = 1.0
LAMBDA_COR = 0.1
# meta layout (f32): [lg 40 | pp 40 | pn 40 | pick 4] = 124 columns
MW = ST * (3 * NUM_CLASSES + 1)
QCHUNK = 512  # PSUM row width for the PE global reduction
# size-rank chunk -> supertile slot (2nd-largest first, smallest last)
STMAP = (1, 0, 2, 3)

LAST_RESULTS = None  # BassKernelResults of the most recent run (for test harness)


def _build_nc(pads: tuple) -> bass.Bass:
    """Build the SPMD Bass program (identical on all 8 cores)."""
    F = sum(pads)
    Pmax = max(int(p) for p in pads)
    offs = np.concatenate([[0], np.cumsum(pads)]).astype(int)
    nc = bacc.Bacc(None, num_devices=NCORES)

    xp_d = nc.declare_dram_parameter("xp", [128, F], F16, isOutput=False)
    xn_d = nc.declare_dram_parameter("xn", [128, F], F16, isOutput=False)
    mt_d = nc.declare_dram_parameter("mt", [128, MW], F32, isOutput=False)
    out_d = nc.declare_dram_parameter("out", [128, 8], F32, isOutput=True)
    kl_d = nc.declare_dram_parameter("klrow", [1, QCHUNK], F32, isOutput=True)

    ones_np = np.ones((128, 2), np.float16)
    ones_np[:, 1] = -1.0
    ones_d = nc.inline_tensor(ones_np, name="pmones")

    with tile.TileContext(nc) as tc:
        with (
            tc.tile_pool(name="scr", bufs=2) as spool,
            tc.tile_pool(name="small", bufs=2) as mpool,
            tc.tile_pool(name="persist", bufs=1) as ppool,
            tc.tile_pool(name="psum", bufs=1, space="PSUM") as pspool,
        ):
            # persistent tensors
            xp_t = ppool.tile([128, F], F16)
            xn_t = ppool.tile([128, F], F16)
            slu_t = ppool.tile([128, 3 * F], F16)   # [spH | snH | u2] per ST
            lgu_t = ppool.tile([128, 3 * F], F16)   # ln(2*slu + b)
            mt_t = ppool.tile([128, MW], F32)
            out_t = ppool.tile([128, 8], F32)
            eps_t = ppool.tile([128, 1], F32)
            nc.vector.memset(eps_t[:], EPS * SCL)
            ones_t = ppool.tile([128, 2], F16)

            SP = ppool.tile([128, ST], F32)   # sum xp
            SN = ppool.tile([128, ST], F32)   # sum xn
            RPS = ppool.tile([128, ST], F32)  # SCL/2 rides the norm imm
            RNS = ppool.tile([128, ST], F32)
            sp0h = ppool.tile([128, 2], F32)  # split first-ST xp sum halves

            qsum = pspool.tile([1, QCHUNK], F32)
            n_mm = sum(
                len(range(0, 2 * int(p), QCHUNK)) + len(range(0, int(p), QCHUNK))
                for p in pads
            )
            mm_i = [0]

            def q_reduce(buf, base, width, neg):
                w = ones_t[:, 1:2] if neg else ones_t[:, 0:1]
                for c0 in range(0, width, QCHUNK):
                    c1 = min(c0 + QCHUNK, width)
                    nc.tensor.matmul(
                        qsum[:, 0 : c1 - c0],
                        lhsT=w,
                        rhs=buf[:, base + c0 : base + c1],
                        start=(mm_i[0] == 0),
                        stop=(mm_i[0] == n_mm - 1),
                    )
                    mm_i[0] += 1

            # DMA order: meta, then xn before xp per supertile (the xn fold
            # chain gates the first norms), first xp chunk split in halves
            nc.sync.dma_start(mt_t[:], mt_d[:])
            h0 = int(pads[0]) // 2
            nc.sync.dma_start(xn_t[:, 0 : int(pads[0])], xn_d[:, 0 : int(pads[0])])
            nc.sync.dma_start(xp_t[:, 0:h0], xp_d[:, 0:h0])
            nc.sync.dma_start(xp_t[:, h0 : int(pads[0])], xp_d[:, h0 : int(pads[0])])
            for s in range(1, ST):
                a, b = int(offs[s]), int(offs[s + 1])
                nc.sync.dma_start(xn_t[:, a:b], xn_d[:, a:b])
                nc.sync.dma_start(xp_t[:, a:b], xp_d[:, a:b])
            nc.sync.dma_start(ones_t[:], ones_d[:])

            # ---- CE partials + MSE (small, run during the DMA fill) ----
            lg = mt_t[:, 0 : ST * NUM_CLASSES]
            ppb = mt_t[:, ST * NUM_CLASSES : 2 * ST * NUM_CLASSES]
            pnb = mt_t[:, 2 * ST * NUM_CLASSES : 3 * ST * NUM_CLASSES]
            e_t = mpool.tile([128, ST * NUM_CLASSES], F32, tag="e")
            nc.scalar.activation(e_t[:], lg, ACTF.Exp)
            # dummy Ln: pulls the natural_log table load into the DMA fill
            # window instead of stalling the first big Ln pass
            warm = mpool.tile([128, 1], F32, tag="warm")
            nc.scalar.activation(warm[:], eps_t[:], ACTF.Ln)
            for s in range(ST):
                nc.vector.reduce_sum(
                    out_t[:, 2 + s : 3 + s],
                    e_t[:, s * NUM_CLASSES : (s + 1) * NUM_CLASSES],
                    axis=AX.X,
                )
            d_t = mpool.tile([128, ST * NUM_CLASSES], F32, tag="d")
            nc.vector.scalar_tensor_tensor(
                d_t[:], ppb, -1.0, pnb, op0=ALU.add, op1=ALU.add
            )
            d2_t = mpool.tile([128, ST * NUM_CLASSES], F32, tag="d2")
            nc.vector.scalar_tensor_tensor(
                d2_t[:], d_t[:], 1.0, d_t[:], op0=ALU.bypass, op1=ALU.mult,
                accum_out=out_t[:, 1:2],
            )

            def emit_sums(s):
                a, b = int(offs[s]), int(offs[s + 1])
                P = int(pads[s])
                # sum xp on Scalar (first ST in halves so it starts earlier)
                if s == 0:
                    scpa = spool.tile([128, Pmax], F16, tag="scp")
                    nc.scalar.activation(
                        scpa[:, 0:h0], xp_t[:, 0:h0], ACTF.Copy,
                        accum_out=sp0h[:, 0:1],
                    )
                    scpb = spool.tile([128, Pmax], F16, tag="scp")
                    nc.scalar.activation(
                        scpb[:, 0 : P - h0], xp_t[:, h0:P], ACTF.Copy,
                        accum_out=sp0h[:, 1:2],
                    )
                    nc.vector.tensor_tensor(
                        SP[:, 0:1], sp0h[:, 0:1], sp0h[:, 1:2], op=ALU.add
                    )
                else:
                    scp = spool.tile([128, Pmax], F16, tag="scp")
                    nc.scalar.activation(
                        scp[:, 0:P], xp_t[:, a:b], ACTF.Copy,
                        accum_out=SP[:, s : s + 1],
                    )
                # sum xn on Vector: fold halves twice (2x rate), short accum
                h = P // 2
                q = h // 2
                fold = spool.tile([128, Pmax // 2], F16, tag="fold")
                nc.vector.tensor_tensor(
                    fold[:, 0:h], xn_t[:, a : a + h], xn_t[:, a + h : b], op=ALU.add
                )
                fold2 = spool.tile([128, Pmax // 4], F16, tag="fold2")
                nc.vector.tensor_tensor(
                    fold2[:, 0:q], fold[:, 0:q], fold[:, q:h], op=ALU.add
                )
                fscr = spool.tile([128, Pmax // 4], F16, tag="fold2")
                nc.vector.tensor_scalar(
                    fscr[:, 0:q], fold2[:, 0:q], 1.0, 0.0, op0=ALU.mult, op1=ALU.add,
                    accum_out=SN[:, s : s + 1],
                )
                # rp = 1/(sum+eps); SCL/2 rides the norm pass imm
                spe = mpool.tile([128, 1], F32, tag="spe")
                nc.vector.tensor_scalar(
                    spe[:], SP[:, s : s + 1], EPS, 0.0, op0=ALU.add, op1=ALU.bypass
                )
                nc.vector.reciprocal(RPS[:, s : s + 1], spe[:])
                sne = mpool.tile([128, 1], F32, tag="sne")
                nc.vector.tensor_scalar(
                    sne[:], SN[:, s : s + 1], EPS, 0.0, op0=ALU.add, op1=ALU.bypass
                )
                nc.vector.reciprocal(RNS[:, s : s + 1], sne[:])

            def emit_norms(s):
                a, b = int(offs[s]), int(offs[s + 1])
                P = int(pads[s])
                g = 3 * a
                nc.vector.tensor_scalar(
                    slu_t[:, g : g + P], xp_t[:, a:b], RPS[:, s : s + 1], SCL / 2,
                    op0=ALU.mult, op1=ALU.mult,
                )
                nc.vector.tensor_scalar(
                    slu_t[:, g + P : g + 2 * P], xn_t[:, a:b], RNS[:, s : s + 1],
                    SCL / 2, op0=ALU.mult, op1=ALU.mult,
                )
                nc.vector.tensor_tensor(
                    slu_t[:, g + 2 * P : g + 3 * P], slu_t[:, g : g + P],
                    slu_t[:, g + P : g + 2 * P], op=ALU.add,
                )

            def emit_log2(s):
                # ln over the contiguous [spH | snH] block; runs while the
                # Vector engine builds u2
                a = int(offs[s])
                P = int(pads[s])
                g = 3 * a
                nc.scalar.activation(
                    lgu_t[:, g : g + 2 * P], slu_t[:, g : g + 2 * P], ACTF.Ln,
                    bias=eps_t[:], scale=2.0,
                )

            def emit_logu(s):
                a = int(offs[s])
                P = int(pads[s])
                g = 3 * a
                nc.scalar.activation(
                    lgu_t[:, g + 2 * P : g + 3 * P], slu_t[:, g + 2 * P : g + 3 * P],
                    ACTF.Ln, bias=eps_t[:], scale=2.0,
                )

            def emit_prod2(s):
                a = int(offs[s])
                P = int(pads[s])
                g = 3 * a
                pr = spool.tile([128, 3 * Pmax], F16, tag="pr")
                nc.vector.tensor_tensor(
                    pr[:, 0 : 2 * P], slu_t[:, g : g + 2 * P],
                    lgu_t[:, g : g + 2 * P], op=ALU.mult,
                )
                q_reduce(pr, 0, 2 * P, neg=False)

            def emit_produ(s):
                a = int(offs[s])
                P = int(pads[s])
                g = 3 * a
                pu = spool.tile([128, Pmax], F16, tag="pu")
                nc.vector.tensor_tensor(
                    pu[:, 0:P], slu_t[:, g + 2 * P : g + 3 * P],
                    lgu_t[:, g + 2 * P : g + 3 * P], op=ALU.mult,
                )
                q_reduce(pu, 0, P, neg=True)

            # software-pipelined emission
            emit_sums(0)
            emit_sums(1)
            emit_norms(0)
            emit_log2(0)
            emit_logu(0)
            emit_sums(2)
            emit_norms(1)
            emit_prod2(0)
            emit_produ(0)
            emit_log2(1)
            emit_logu(1)
            emit_sums(3)
            emit_norms(2)
            emit_prod2(1)
            emit_produ(1)
            emit_log2(2)
            emit_logu(2)
            emit_norms(3)
            emit_prod2(2)
            emit_produ(2)
            emit_log2(3)
            emit_logu(3)
            emit_prod2(3)
            emit_produ(3)

            # ---- nz count + output ----
            nzc = mpool.tile([128, ST], F32, tag="nzc")
            nc.vector.tensor_scalar(
                nzc[:], SP[:], 0.0, 0.0, op0=ALU.is_gt, op1=ALU.bypass
            )
            f0 = mpool.tile([128, ST], F32, tag="f0")
            nc.vector.tensor_scalar(
                f0[:], nzc[:], 1.0, 0.0, op0=ALU.mult, op1=ALU.add,
                accum_out=out_t[:, 0:1],
            )
            nc.vector.tensor_copy(out_t[:, 6:7], eps_t[:])
            nc.vector.tensor_copy(out_t[:, 7:8], eps_t[:])
            nc.sync.dma_start(out_d[:], out_t[:])
            klrow = ppool.tile([1, QCHUNK], F32)
            nc.vector.tensor_copy(klrow[:], qsum[:])
            nc.sync.dma_start(kl_d[:], klrow[:])

    nc.finalize()
    return nc


def _pack_host(score_pos, score_neg, batch, logits, pp, pn, targets):
    """Sort graphs by size, assign size-rank chunks to supertiles via STMAP,
    scatter nodes into per-core [128, F] fp16 blocks with tight per-ST pads."""
    counts = np.bincount(batch, minlength=NUM_GRAPHS)
    order_sz = np.argsort(-counts, kind="stable")  # rank -> graph id
    chunk_pad = [
        int(np.ceil(max(int(counts[order_sz[c * 1024 : (c + 1) * 1024]].max()), 64) / 64) * 64)
        for c in range(ST)
    ]
    pads = [0] * ST
    for c in range(ST):
        pads[STMAP[c]] = chunk_pad[c]
    pads = tuple(pads)
    F = sum(pads)
    offs = np.concatenate([[0], np.cumsum(pads)]).astype(np.int64)
    stmap = np.asarray(STMAP, np.int64)

    rank_of = np.empty(NUM_GRAPHS, np.int64)
    rank_of[order_sz] = np.arange(NUM_GRAPHS)

    n = batch.shape[0]
    order = np.argsort(batch, kind="stable")
    bs = batch[order]
    starts = np.zeros(NUM_GRAPHS, np.int64)
    starts[1:] = np.cumsum(counts)[:-1]
    pos = np.arange(n, dtype=np.int64) - starts[bs]

    r = rank_of[bs]
    s_arr = stmap[r >> 10]
    q = r & 1023
    c_arr = q >> 7
    p_arr = q & 127
    flat = (c_arr * 128 + p_arr) * F + offs[s_arr] + pos

    xp = np.zeros(NCORES * 128 * F, np.float16)
    xn = np.zeros(NCORES * 128 * F, np.float16)
    xp[flat] = score_pos[order].astype(np.float16)
    xn[flat] = score_neg[order].astype(np.float16)
    xp = xp.reshape(NCORES, 128, F)
    xn = xn.reshape(NCORES, 128, F)

    # meta: [lg 40 | pp 40 | pn 40 | pick 4] per partition, f32
    ranks = np.arange(NUM_GRAPHS)
    g_at = order_sz[ranks]
    s_g = stmap[ranks >> 10]
    q_g = ranks & 1023
    c_g = q_g >> 7
    p_g = q_g & 127
    mt = np.zeros((NCORES, 128, MW), np.float32)
    C = NUM_CLASSES
    picked = logits[np.arange(NUM_GRAPHS), targets.astype(np.int64)]
    for s in range(ST):
        m = s_g == s
        mt[c_g[m], p_g[m], s * C : (s + 1) * C] = logits[g_at[m]]
        mt[c_g[m], p_g[m], ST * C + s * C : ST * C + (s + 1) * C] = pp[g_at[m]]
        mt[c_g[m], p_g[m], 2 * ST * C + s * C : 2 * ST * C + (s + 1) * C] = pn[g_at[m]]
        mt[c_g[m], p_g[m], 3 * ST * C + s] = picked[g_at[m]]
    pick_sum = float(picked.sum(dtype=np.float64))
    return xp, xn, mt, pads, pick_sum


_NC_CACHE: dict = {}


def kernel(logits_pos, probs_pos, probs_neg, score_pos, score_neg, targets, batch):
    global LAST_RESULTS
    logits_pos = np.asarray(logits_pos, np.float32)
    probs_pos = np.asarray(probs_pos, np.float32)
    probs_neg = np.asarray(probs_neg, np.float32)
    score_pos = np.asarray(score_pos, np.float32)
    score_neg = np.asarray(score_neg, np.float32)
    targets = np.asarray(targets)
    batch = np.asarray(batch)

    xp, xn, mt, pads, pick_sum = _pack_host(
        score_pos, score_neg, batch, logits_pos, probs_pos, probs_neg, targets
    )

    if pads not in _NC_CACHE:
        _NC_CACHE[pads] = _build_nc(pads)
    nc = _NC_CACHE[pads]

    in_maps = [{"xp": xp[c], "xn": xn[c], "mt": mt[c]} for c in range(NCORES)]
    trace = bool(int(os.environ.get("KERNEL_TRACE", "0")))
    res = run_bass_kernel_spmd(nc, in_maps, list(range(NCORES)), trace=trace)
    LAST_RESULTS = res

    # unshard: sum the per-core partials, finish the formula on the host
    nz_sum = 0.0
    mse_sum = 0.0
    lse_sum = 0.0
    R = 0.0
    for c in range(NCORES):
        o = np.asarray(res.results[c]["out"], np.float64).reshape(128, 8)
        nz_sum += o[:, 0].sum()
        mse_sum += o[:, 1].sum()
        lse_sum += np.log(o[:, 2:6]).sum()
        R += np.asarray(res.results[c]["klrow"], np.float64).sum()
    kl_sum = 2.0 * R / SCL + 2.0 * np.log(2.0) * nz_sum
    ce_sum = lse_sum - pick_sum
    js = 0.5 * kl_sum / nz_sum
    l_train = ce_sum / NUM_GRAPHS
    mse = mse_sum / (NUM_GRAPHS * NUM_CLASSES)
    l_cor = ALPHA * js + BETA * mse
    l_total = l_train + LAMBDA_COR * l_cor
    return (np.float32(l_total), np.float32(l_train), np.float32(l_cor))
